# revision 5
# baseline (speedup 1.0000x reference)
"""Trainium2 Bass kernel for the 6-node GCN classification model.

Math: each GCN layer is h' = relu(A @ h @ W^T + b) on [B, 6, 64], where A is
the 6x6 normalized adjacency (with self loops; fill=1.0 for layers 1-2,
fill=2.0 for layers 3-4).  With the 6 nodes stacked in pairs on the 128 SBUF
partitions (2 nodes x 64 features), the fused per-layer operator A (x) W is a
384x384 block matrix; each nonzero 128x128 block becomes one TensorE matmul
accumulated in PSUM.  Node pairings alternate between two stackings chosen so
the total block count over the 4 layers is the provable minimum (26).

Pipeline per 512-batch group, fully fused in SBUF (x is read from HBM exactly
once, only the [B, 6] sigmoid output is written back):
  DMA x (batch-major, contiguous) -> PE transpose to feature-major stacked
  -> 4 x (block matmuls f32r -> ACT bias+ReLU) -> DVE residual add
  -> fc-head matmuls -> ACT sigmoid -> PE transpose back -> DMA out.

Sharding: pure data parallel over the batch dim across the 8 NeuronCores.
"""

import math
from contextlib import ExitStack

import numpy as np

N_CORES = 8
BATCH = 131072
PER_CORE = BATCH // N_CORES  # 16384
NN = 6
FEAT = 64
GROUP = 512
N_GROUPS = PER_CORE // GROUP  # 32

SRC = [1, 2, 0, 2, 1, 3, 2, 4, 3, 5, 3, 4]
DST = [0, 0, 1, 1, 2, 2, 3, 3, 4, 4, 5, 5]

# Node pair stackings per layer boundary (chain start == chain end so the
# residual/fc read the same stacking the input transposes produce).
S_A = [(0, 1), (2, 3), (4, 5)]
S_B = [(0, 5), (1, 2), (3, 4)]
CHAIN = [S_A, S_B, S_A, S_B, S_A]  # layer l maps CHAIN[l] -> CHAIN[l+1]


def _gcn_A(fill: float) -> np.ndarray:
    """Dense [6, 6] aggregation matrix A[dst, src] incl. weighted self loops."""
    src = SRC + list(range(NN))
    dst = DST + list(range(NN))
    w = [1.0] * len(SRC) + [fill] * NN
    deg = np.zeros(NN, np.float64)
    for s, d, ww in zip(src, dst, w):
        deg[d] += ww
    dinv = np.where(deg > 0, 1.0 / np.sqrt(deg), 0.0)
    A = np.zeros((NN, NN), np.float64)
    for s, d, ww in zip(src, dst, w):
        A[d, s] += dinv[s] * ww * dinv[d]
    return A


def _block_plan():
    """Static plan: for each layer, the nonzero (out_tile, in_tile) blocks.

    Returns [layer][out_tile] -> list of in_tile indices, using the support of
    A (same for both fill values)."""
    S = np.zeros((NN, NN), bool)
    for s, d in zip(SRC, DST):
        S[d, s] = True
    for i in range(NN):
        S[i, i] = True
    plan = []
    for layer in range(4):
        inp, outp = CHAIN[layer], CHAIN[layer + 1]
        lplan = []
        for (n0, n1) in outp:
            js = []
            for j, (m0, m1) in enumerate(inp):
                if S[n0, m0] or S[n0, m1] or S[n1, m0] or S[n1, m1]:
                    js.append(j)
            lplan.append(js)
        plan.append(lplan)
    return plan


BLOCK_PLAN = _block_plan()
N_BLOCKS = sum(len(js) for lp in BLOCK_PLAN for js in lp)  # 26


def build_consts(W, b, fc_w, fc_b):
    """Host-side constant tensors fed to the device as DRAM inputs.

    W: list of 4 [64, 64] arrays; b: list of 4 [64]; fc_w [6, 64]; fc_b [6].
    """
    A = [_gcn_A(1.0), _gcn_A(1.0), _gcn_A(2.0), _gcn_A(2.0)]
    wblk = np.zeros((N_BLOCKS, 128, 128), np.float32)
    k = 0
    for layer in range(4):
        inp, outp = CHAIN[layer], CHAIN[layer + 1]
        Wt = W[layer].T.astype(np.float64)  # [f, g] = W[g, f]
        for i, (n0, n1) in enumerate(outp):
            for j in BLOCK_PLAN[layer][i]:
                m0, m1 = inp[j]
                blk = np.zeros((128, 128), np.float64)
                for dj, m in enumerate((m0, m1)):
                    for do, n in enumerate((n0, n1)):
                        a = A[layer][n, m]
                        if a != 0.0:
                            blk[dj * 64:(dj + 1) * 64, do * 64:(do + 1) * 64] = a * Wt
                wblk[k] = blk.astype(np.float32)
                k += 1
    assert k == N_BLOCKS

    bias = np.zeros((4, 128, 1), np.float32)
    for layer in range(4):
        bias[layer, :, 0] = np.tile(b[layer], 2)

    fcw = np.zeros((3, 128, NN), np.float32)
    for i, (n0, n1) in enumerate(CHAIN[4]):
        for do, n in enumerate((n0, n1)):
            fcw[i, do * 64:(do + 1) * 64, n] = fc_w[n]

    return {
        "wblk": wblk,
        "bias": bias,
        "fcw": fcw,
        "fcb": fc_b.astype(np.float32).reshape(NN, 1),
        "eye128": np.eye(128, dtype=np.float32),
        "eye6": np.eye(NN, dtype=np.float32),
    }


def build_program(repeats: int = 1):
    """Build + schedule + compile the Bass/Tile program. Returns nc."""
    import concourse.tile as tile
    import concourse.mybir as mybir
    from concourse import bacc

    f32 = mybir.dt.float32
    f32r = mybir.dt.float32r
    Relu = mybir.ActivationFunctionType.Relu
    Sigmoid = mybir.ActivationFunctionType.Sigmoid

    nc = bacc.Bacc("TRN2", target_bir_lowering=False, debug=False,
                   num_devices=N_CORES)

    x_ap = nc.dram_tensor("x", [PER_CORE, NN * FEAT], f32,
                          kind="ExternalInput").ap()
    y_ap = nc.dram_tensor("y", [PER_CORE, NN], f32,
                          kind="ExternalOutput").ap()
    wblk_ap = nc.dram_tensor("wblk", [N_BLOCKS, 128, 128], f32r,
                             kind="ExternalInput").ap()
    bias_ap = nc.dram_tensor("bias", [4, 128, 1], f32,
                             kind="ExternalInput").ap()
    fcw_ap = nc.dram_tensor("fcw", [3, 128, NN], f32r,
                            kind="ExternalInput").ap()
    fcb_ap = nc.dram_tensor("fcb", [NN, 1], f32, kind="ExternalInput").ap()
    eye128_ap = nc.dram_tensor("eye128", [128, 128], f32,
                               kind="ExternalInput").ap()
    eye6_ap = nc.dram_tensor("eye6", [NN, NN], f32,
                             kind="ExternalInput").ap()

    SB = GROUP // 128  # 4 batch sub-tiles per group

    with tile.TileContext(nc) as tc, ExitStack() as ctx:
        cpool = ctx.enter_context(tc.tile_pool(name="consts", bufs=1))
        # Block weights
        wt = []
        for k in range(N_BLOCKS):
            t = cpool.tile([128, 128], f32r, tag=f"w{k}")
            nc.sync.dma_start(t[:], wblk_ap[k])
            wt.append(t)
        bt = []
        for layer in range(4):
            t = cpool.tile([128, 1], f32, tag=f"b{layer}")
            nc.sync.dma_start(t[:], bias_ap[layer])
            bt.append(t)
        fct = []
        for i in range(3):
            t = cpool.tile([128, NN], f32r, tag=f"fc{i}")
            nc.sync.dma_start(t[:], fcw_ap[i])
            fct.append(t)
        fcbt = cpool.tile([NN, 1], f32, tag="fcb")
        nc.sync.dma_start(fcbt[:], fcb_ap[:])
        eye128 = cpool.tile([128, 128], f32, tag="eye128")
        nc.sync.dma_start(eye128[:], eye128_ap[:])
        eye6 = cpool.tile([NN, NN], f32, tag="eye6")
        nc.sync.dma_start(eye6[:], eye6_ap[:])

        p_xb = ctx.enter_context(tc.tile_pool(name="xb", bufs=3))
        p_xs = ctx.enter_context(tc.tile_pool(name="xs", bufs=2))
        p_h = ctx.enter_context(tc.tile_pool(name="h", bufs=4))
        p_r = ctx.enter_context(tc.tile_pool(name="r", bufs=2))
        p_sig = ctx.enter_context(tc.tile_pool(name="sig", bufs=2))
        p_ob = ctx.enter_context(tc.tile_pool(name="ob", bufs=2))
        p_px = ctx.enter_context(tc.tile_pool(name="px", bufs=2, space="PSUM"))
        p_ph = ctx.enter_context(tc.tile_pool(name="ph", bufs=4, space="PSUM"))
        p_pfc = ctx.enter_context(tc.tile_pool(name="pfc", bufs=1, space="PSUM"))
        p_pot = ctx.enter_context(tc.tile_pool(name="pot", bufs=1, space="PSUM"))

        def group_body(g):
            # Load [512, 384] rows batch-major: partition = batch % 128.
            xb = p_xb.tile([128, SB * NN * FEAT], f32, tag="xb")
            nc.sync.dma_start(
                xb[:].rearrange("p (s f) -> p s f", s=SB),
                x_ap[g * GROUP:(g + 1) * GROUP, :].rearrange(
                    "(s p) f -> p s f", p=128),
            )
            # Transpose to feature-major stacked (pairs = CHAIN[0]).
            xs = []
            for j in range(3):
                xt = p_px.tile([128, GROUP], f32, tag="xt")
                for s in range(SB):
                    nc.tensor.transpose(
                        xt[:, s * 128:(s + 1) * 128],
                        xb[:, s * NN * FEAT + j * 128:
                           s * NN * FEAT + (j + 1) * 128],
                        eye128[:],
                    )
                xsj = p_xs.tile([128, GROUP], f32r, tag=f"xs{j}")
                nc.vector.tensor_copy(out=xsj[:], in_=xt[:])
                xs.append(xsj)

            h = xs
            k = 0
            for layer in range(4):
                hn = []
                for i in range(3):
                    ps = p_ph.tile([128, GROUP], f32, tag="ph")
                    js = BLOCK_PLAN[layer][i]
                    for bi, j in enumerate(js):
                        nc.tensor.matmul(
                            ps[:],
                            lhsT=wt[k][:],
                            rhs=h[j][:],
                            start=(bi == 0),
                            stop=(bi == len(js) - 1),
                        )
                        k += 1
                    ht = p_h.tile([128, GROUP], f32r, tag=f"h{i}")
                    nc.scalar.activation(ht[:], ps[:], Relu, bias=bt[layer][:])
                    hn.append(ht)
                h = hn
            assert k == N_BLOCKS

            # Residual + fc heads + sigmoid.
            psfc = p_pfc.tile([NN, GROUP], f32, tag="pfc")
            for i in range(3):
                ri = p_r.tile([128, GROUP], f32r, tag=f"r{i}")
                nc.vector.tensor_add(out=ri[:], in0=h[i][:], in1=xs[i][:])
                nc.tensor.matmul(
                    psfc[:],
                    lhsT=fct[i][:],
                    rhs=ri[:],
                    start=(i == 0),
                    stop=(i == 2),
                )
            sig = p_sig.tile([NN, GROUP], f32, tag="sig")
            nc.scalar.activation(sig[:], psfc[:], Sigmoid, bias=fcbt[:])

            # Back to batch-major and store.
            ot = p_pot.tile([128, SB * NN], f32, tag="ot")
            for s in range(SB):
                nc.tensor.transpose(
                    ot[:, s * NN:(s + 1) * NN],
                    sig[:, s * 128:(s + 1) * 128],
                    eye6[:],
                )
            ob = p_ob.tile([128, SB * NN], f32, tag="ob")
            nc.vector.tensor_copy(out=ob[:], in_=ot[:])
            nc.sync.dma_start(
                y_ap[g * GROUP:(g + 1) * GROUP, :].rearrange(
                    "(s p) n -> p s n", p=128),
                ob[:].rearrange("p (s n) -> p s n", s=SB),
            )

        if repeats == 1:
            for g in range(N_GROUPS):
                group_body(g)
        else:
            with tc.For_i(0, repeats):
                for g in range(N_GROUPS):
                    group_body(g)

    nc.compile()
    return nc


class Runner:
    """Compiled program + cached jitted PJRT executable over the 8 cores.

    Mirrors concourse.bass2jax.run_bass_via_pjrt, but keeps the jitted
    callable and accepts device-resident inputs so repeated timed calls do
    not re-trace or re-transfer."""

    def __init__(self, nc):
        import jax
        import numpy as _np
        from jax.sharding import Mesh, PartitionSpec, NamedSharding
        from jax.experimental.shard_map import shard_map
        import concourse.mybir as mybir
        from concourse import bass2jax

        bass2jax.install_neuronx_cc_hook()
        self.nc = nc
        assert nc.dbg_addr is None
        partition_name = (nc.partition_id_tensor.name
                          if nc.partition_id_tensor else None)

        in_names, out_names, out_avals, zero_outs = [], [], [], []
        for alloc in nc.m.functions[0].allocations:
            if not isinstance(alloc, mybir.MemoryLocationSet):
                continue
            name = alloc.memorylocations[0].name
            if alloc.kind == "ExternalInput":
                if name == partition_name:
                    continue
                in_names.append(name)
            elif alloc.kind == "ExternalOutput":
                shape = tuple(alloc.tensor_shape)
                dtype = mybir.dt.np(alloc.dtype)
                out_names.append(name)
                out_avals.append(jax.core.ShapedArray(shape, dtype))
                zero_outs.append(_np.zeros(shape, dtype))
        self.in_names = list(in_names)
        self.out_names = out_names
        self.out_avals = out_avals
        self.zero_outs = zero_outs
        n_params = len(in_names)
        n_outs = len(out_avals)
        all_in_names = in_names + out_names
        if partition_name is not None:
            all_in_names = all_in_names + [partition_name]
        donate = tuple(range(n_params, n_params + n_outs))

        def _body(*args):
            operands = list(args)
            if partition_name is not None:
                operands.append(bass2jax.partition_id_tensor())
            outs = bass2jax._bass_exec_p.bind(
                *operands,
                out_avals=tuple(out_avals),
                in_names=tuple(all_in_names),
                out_names=tuple(out_names),
                lowering_input_output_aliases=(),
                sim_require_finite=True,
                sim_require_nnan=True,
                nc=nc,
            )
            return tuple(outs)

        devices = jax.devices()[:N_CORES]
        self.mesh = Mesh(_np.asarray(devices), ("core",))
        self.sharding = NamedSharding(self.mesh, PartitionSpec("core"))
        in_specs = (PartitionSpec("core"),) * (n_params + n_outs)
        out_specs = (PartitionSpec("core"),) * n_outs
        self.jitted = jax.jit(
            shard_map(_body, mesh=self.mesh, in_specs=in_specs,
                      out_specs=out_specs, check_rep=False),
            donate_argnums=donate,
            keep_unused=True,
        )
        self._jax = jax

    def put_inputs(self, in_maps):
        """in_maps: list of N_CORES dicts name->np.  Returns device arrays."""
        import numpy as _np
        concat = [
            _np.concatenate([_np.asarray(m[name]) for m in in_maps], axis=0)
            for name in self.in_names
        ]
        return [self._jax.device_put(a, self.sharding) for a in concat]

    def run(self, dev_inputs):
        jax = self._jax
        zeros = [
            jax.device_put(
                self._jax.numpy.zeros((N_CORES * z.shape[0], *z.shape[1:]),
                                      z.dtype),
                self.sharding)
            for z in self.zero_outs
        ]
        outs = self.jitted(*dev_inputs, *zeros)
        outs = [self._jax.block_until_ready(o) for o in outs]
        return {
            name: outs[i]
            for i, name in enumerate(self.out_names)
        }


_RUNNERS = {}


def get_runner(repeats: int = 1) -> Runner:
    if repeats not in _RUNNERS:
        _RUNNERS[repeats] = Runner(build_program(repeats))
    return _RUNNERS[repeats]


def _make_in_maps(inputs):
    x = np.ascontiguousarray(np.asarray(inputs["x"], np.float32))
    assert x.shape == (BATCH, NN, FEAT)
    consts = build_consts(
        W=[np.asarray(inputs[f"W{i+1}"], np.float32) for i in range(4)],
        b=[np.asarray(inputs[f"b{i+1}"], np.float32) for i in range(4)],
        fc_w=np.asarray(inputs["fc_w"], np.float32),
        fc_b=np.asarray(inputs["fc_b"], np.float32),
    )
    x_sh = x.reshape(N_CORES, PER_CORE, NN * FEAT)
    return [{"x": x_sh[c], **consts} for c in range(N_CORES)]


def kernel(**inputs) -> np.ndarray:
    runner = get_runner(repeats=1)
    dev = runner.put_inputs(_make_in_maps(inputs))
    out = runner.run(dev)
    y = np.asarray(out["y"]).reshape(BATCH, NN)
    return y
